# revision 2
# baseline (speedup 1.0000x reference)
"""DeformableAttention on 8 Trainium2 NeuronCores via a hand-written Bass/Tile kernel.

Data-parallel over batch: one batch item per core, identical SPMD program.
All weight-derived tensors are baked into the NEFF as Const (inline_tensor);
the only runtime input is xT, the only output y.

v2 changes vs baseline:
- relative-position bias table host-expanded to [8h, 8jt, 128, 1024] bf16 so
  the per-(head, j-tile) load is one fully contiguous 256KB DMA (the old
  Toeplitz gather moved 16MB/core in 64B chunks, far below DMA line rate).
- conv taps and attention bias multiplies split across DVE and GPSIMD(Pool).
- LN rsqrt and softmax denominators use Rsqrt/Reciprocal activation tables
  (1 op) instead of Sqrt/Ln/Exp chains (3 ops).
- offset tanh + tent ops batched across the y/x axes ([2,N] / [64,N] tiles).
- optional `reps` hardware loop (tc.For_i) repeating the whole body for
  device-time measurement with dispatch overhead cancelled.
"""
import hashlib
import numpy as np
import ml_dtypes
import concourse.bass as bass
import concourse.bacc as bacc
import concourse.mybir as mybir
import concourse.tile as tile

F32 = mybir.dt.float32
BF16 = mybir.dt.bfloat16
AF = mybir.ActivationFunctionType
OP = mybir.AluOpType
PSUM = bass.MemorySpace.PSUM

H = 32
N = 1024
DIM = 256
EPS = 1e-5


def build(nc, consts, reps=1, dbg=False):
    xT_d = nc.dram_tensor("xT", [2, 128, N], BF16, kind="ExternalInput")
    y_d = nc.dram_tensor("y", [8, 128, DIM], F32, kind="ExternalOutput")
    dbg_d = {}
    if dbg:
        for nm, shp in [("qT", [2, 128, N]), ("acc", [2, 128, N]),
                        ("syx", [2, 2, N]), ("ts2", [2, 64, N]),
                        ("S0", [128, N]), ("xsT", [2, 128, N]),
                        ("kT", [2, 128, N]), ("es0", [128, N]),
                        ("eb0", [128, N]), ("ns", [2, 128, N])]:
            dbg_d[nm] = nc.dram_tensor("dbg_" + nm, shp, F32, kind="ExternalOutput")

    def cin(name):
        return nc.inline_tensor(np.ascontiguousarray(consts[name]), name=name)

    Wq_d = cin("Wq")      # [2,128,DIM] bf16
    Wk_d = cin("Wk")
    Wv_d = cin("Wv")
    Wo_d = cin("Wo")
    woff_d = cin("Woff")  # [128,2] f32
    diagw_d = cin("diagw")  # [128, 25*128] bf16: tap t cols t*128:(t+1)*128 = diag(conv_w[:,t])
    convb_d = cin("convb")  # [128,1] f32
    cbm_d = cin("cbmean")   # [1,1] f32
    lng_d = cin("lng")
    lnb_d = cin("lnb")
    npy64_d = cin("npy64")  # [64,1] f32
    eptx_d = cin("Eptx")    # [64, 1152] bf16: rows 0:32 Ept, rows 32:64 cols 1024:1152 Epx
    tyxb_d = cin("tyxb")    # [2,N] f32
    o128_d = cin("inv128")  # [128,1] f32
    o32r_d = cin("ones32r")  # [1,32] f32
    o128r_d = cin("ones128r")  # [1,128] f32
    sel2_d = cin("sel2")    # [2,64] f32
    # expanded bias table: runtime input (device-resident, uploaded once),
    # NOT inline — a 16MB Const in the NEFF costs ~120ms per dispatch through
    # the axon relay (per-call cost scales with executable size).
    ebx_d = nc.dram_tensor("ebx", [8, 8, 128, N], BF16, kind="ExternalInput")
    bout_d = cin("boutb")   # [128,DIM] f32

    with tile.TileContext(nc) as tc:
        import contextlib
        ctx = contextlib.ExitStack()
        with ctx:
            P = ctx.enter_context(tc.tile_pool(name="persist", bufs=1))
            WKB = ctx.enter_context(tc.tile_pool(name="workbig", bufs=1))
            WK = ctx.enter_context(tc.tile_pool(name="work", bufs=1))
            RR2P = ctx.enter_context(tc.tile_pool(name="rr2pool", bufs=2))
            OTS = ctx.enter_context(tc.tile_pool(name="otspool", bufs=4))
            EBP = ctx.enter_context(tc.tile_pool(name="ebpool", bufs=6))
            ESP = ctx.enter_context(tc.tile_pool(name="espool", bufs=4))

            def load(d, shape, dt, tag, sl=None):
                t = P.tile(shape, dt, tag=tag, name=tag)
                nc.sync.dma_start(t[:], d.ap() if sl is None else d.ap()[sl])
                return t

            xTs = [load(xT_d, [128, N], BF16, f"xT{k}", k) for k in range(2)]
            Wqs = [load(Wq_d, [128, DIM], BF16, f"Wq{k}", k) for k in range(2)]
            Wks = [load(Wk_d, [128, DIM], BF16, f"Wk{k}", k) for k in range(2)]
            Wvs = [load(Wv_d, [128, DIM], BF16, f"Wv{k}", k) for k in range(2)]
            Wos = [load(Wo_d, [128, DIM], BF16, f"Wo{k}", k) for k in range(2)]
            woff = load(woff_d, [128, 2], F32, "woff")
            diagw = load(diagw_d, [128, 25 * 128], BF16, "diagw")
            convb = load(convb_d, [128, 1], F32, "convb")
            cbm = load(cbm_d, [1, 1], F32, "cbm")
            lng = load(lng_d, [128, 1], F32, "lng")
            lnb = load(lnb_d, [128, 1], F32, "lnb")
            npy64 = load(npy64_d, [64, 1], F32, "npy64")
            eptx = load(eptx_d, [64, 1152], BF16, "eptx")
            tyxb = load(tyxb_d, [2, N], F32, "tyxb")
            inv128 = load(o128_d, [128, 1], F32, "inv128")
            o32r = load(o32r_d, [1, 32], F32, "o32r")
            o128r = load(o128r_d, [1, 128], F32, "o128r")
            sel2 = load(sel2_d, [2, 64], F32, "sel2")
            boutb = load(bout_d, [128, DIM], F32, "boutb")

            qTs = [P.tile([128, N], BF16, tag=f"qT{k}", name=f"qT{k}") for k in range(2)]
            qti = [P.tile([128, DIM], BF16, tag=f"q{k}", name=f"q{k}") for k in range(8)]
            xss = [P.tile([128, N], BF16, tag=f"xs{k}", name=f"xs{k}") for k in range(2)]
            kTs = [P.tile([128, N], BF16, tag=f"kT{k}", name=f"kT{k}") for k in range(2)]
            v1s = [P.tile([128, 264], BF16, tag=f"v1{k}", name=f"v1{k}") for k in range(8)]
            nss = [P.tile([128, N], BF16, tag=f"ns{k}", name=f"ns{k}") for k in range(2)]
            syx = [P.tile([2, N], F32, tag=f"syx{k}", name=f"syx{k}") for k in range(2)]
            # zero-padded 36x36 q feature maps for the PE depthwise conv;
            # borders zeroed once here (they are never written inside the body).
            # 1332 = 37*36: one spare row so every tap's flat 1152-slice stays
            # in bounds (max read offset 148+1152=1300).
            qps = [P.tile([128, 1332], BF16, tag=f"qp{k}", name=f"qp{k}") for k in range(2)]
            for g in range(2):
                nc.gpsimd.memset(qps[g][:], 0.0)

            def dump(nm, sl, src, cast=None):
                if not dbg:
                    return
                if cast is not None:
                    t = WKB.tile(cast, F32, tag="dbgbig", name="dbgbig")
                    nc.vector.tensor_copy(t[:], src)
                    src = t[:]
                dst = dbg_d[nm].ap() if sl is None else dbg_d[nm].ap()[sl]
                nc.sync.dma_start(dst, src)

            def phases():
                # ---------- phase 1: qT and q ----------
                with tc.tile_pool(name="ps1", bufs=4, space=PSUM) as PS1:
                    for dt in range(2):
                        for ic in range(2):
                            ps = PS1.tile([128, 512], F32, tag="psq")
                            for kt in range(2):
                                nc.tensor.matmul(ps[:], Wqs[kt][:, dt * 128:(dt + 1) * 128],
                                                 xTs[kt][:, ic * 512:(ic + 1) * 512],
                                                 start=kt == 0, stop=kt == 1)
                            nc.vector.tensor_copy(qTs[dt][:, ic * 512:(ic + 1) * 512], ps[:])
                    for it in range(8):
                        ps = PS1.tile([128, 256], F32, tag="psq2")
                        for kt in range(2):
                            nc.tensor.matmul(ps[:], xTs[kt][:, it * 128:(it + 1) * 128],
                                             Wqs[kt][:], start=kt == 0, stop=kt == 1)
                        nc.vector.tensor_copy(qti[it][:], ps[:])
                for dt in range(2):
                    dump("qT", dt, qTs[dt][:], cast=[128, N])

                # ---------- phase 2: conv offset -> LN -> GELU -> offsets ----------
                for g in range(2):
                    # copy q feature map into the interior of the padded tile
                    qg3 = qTs[g][:, :].rearrange("p (y x) -> p y x", y=32)
                    qp3 = qps[g][:, 0:1296].rearrange("p (y x) -> p y x", y=36)
                    nc.vector.tensor_copy(qp3[:, 2:34, 2:34], qg3[:, :, :])
                    # depthwise 5x5 conv as 25 accumulating diag-matmuls on PE.
                    # output is 36-wide padded rows so each tap's rhs is one
                    # flat contiguous slice of the padded input.
                    taps = [(dy, dx) for dy in range(-2, 3) for dx in range(-2, 3)]
                    with tc.tile_pool(name="psconv", bufs=1, space=PSUM) as PSC:
                        accp = PSC.tile([128, 1152], F32, tag="accp", name="accp")
                        for ti, (dy, dx) in enumerate(taps):
                            tap = (dy + 2) * 5 + (dx + 2)
                            st = (dy + 2) * 36 + (dx + 2)
                            for cs_, ce_ in ((0, 512), (512, 1024), (1024, 1152)):
                                nc.tensor.matmul(
                                    accp[:, cs_:ce_],
                                    diagw[:, tap * 128:(tap + 1) * 128],
                                    qps[g][:, st + cs_:st + ce_],
                                    start=ti == 0, stop=ti == 24)
                        acc = WKB.tile([128, N], F32, tag="acc" + str(g), name="acc")
                        nc.vector.tensor_copy(
                            acc[:, :].rearrange("p (y x) -> p y x", y=32),
                            accp[:, :].rearrange("p (y x) -> p y x", y=32)[:, :, 0:32])
                        dump("acc", g, acc[:])
                        sq = WKB.tile([128, N], F32, tag="sq", name="sq")
                        nc.scalar.activation(sq[:], acc[:], AF.Square, bias=convb[:, 0:1], scale=1.0)
                    with tc.tile_pool(name="psln", bufs=1, space=PSUM) as PSL:
                        mu0 = PSL.tile([1, N], F32, tag="mu0")
                        msq = PSL.tile([1, N], F32, tag="msq")
                        for ic in range(2):
                            sl = slice(ic * 512, (ic + 1) * 512)
                            nc.tensor.matmul(mu0[:, sl], inv128[:], acc[:, sl], start=True, stop=True)
                            nc.tensor.matmul(msq[:, sl], inv128[:], sq[:, sl], start=True, stop=True)
                        mu = WK.tile([1, N], F32, tag="mu", name="mu")
                        nc.vector.tensor_scalar(mu[:], mu0[:], cbm[:, 0:1], None, op0=OP.add)
                        r0 = WK.tile([1, N], F32, tag="r0", name="r0")
                        r1 = WK.tile([1, N], F32, tag="r1", name="r1")
                        nc.vector.tensor_tensor(r0[:], mu[:], mu[:], op=OP.mult)
                        nc.vector.scalar_tensor_tensor(r1[:], msq[:], EPS, r0[:],
                                                       op0=OP.add, op1=OP.subtract)
                        ri = WK.tile([1, N], F32, tag="ri", name="ri")
                        nc.vector.reciprocal_approx_fast(ri[:], r1[:])
                        rr = r0
                        nc.scalar.activation(rr[:], ri[:], AF.Sqrt)
                        mur = r1
                        nc.vector.tensor_tensor(mur[:], mu[:], rr[:], op=OP.mult)
                        Rb = PSL.tile([128, N], F32, tag="Rb")
                        Mb = PSL.tile([128, N], F32, tag="Mb")
                        for ic in range(2):
                            sl = slice(ic * 512, (ic + 1) * 512)
                            nc.tensor.matmul(Rb[:, sl], o128r[:], rr[:, sl], start=True, stop=True)
                            nc.tensor.matmul(Mb[:, sl], o128r[:], mur[:, sl], start=True, stop=True)
                        t1 = WKB.tile([128, N], F32, tag="t1", name="t1")
                        nc.vector.scalar_tensor_tensor(t1[:], acc[:], convb[:, 0:1], Rb[:],
                                                       op0=OP.add, op1=OP.mult)
                        on = WKB.tile([128, N], F32, tag="on", name="on")
                        nc.vector.scalar_tensor_tensor(on[:], t1[:], 0.0, Mb[:],
                                                       op0=OP.add, op1=OP.subtract)
                    u = WKB.tile([128, N], F32, tag="u", name="u")
                    nc.vector.tensor_scalar(u[:], on[:], lng[:, 0:1], lnb[:, 0:1],
                                            op0=OP.mult, op1=OP.add)
                    sg = WKB.tile([128, N], F32, tag="sg", name="sg")
                    nc.scalar.activation(sg[:], u[:], AF.Sigmoid, scale=1.702)
                    gl = WKB.tile([128, N], F32, tag="gl", name="gl")
                    nc.gpsimd.tensor_tensor(gl[:], u[:], sg[:], op=OP.mult)
                    with tc.tile_pool(name="psz", bufs=2, space=PSUM) as PSZ:
                        z2 = PSZ.tile([2, N], F32, tag="z2", name="z2")
                        for ic in range(2):
                            sl = slice(ic * 512, (ic + 1) * 512)
                            nc.tensor.matmul(z2[:, sl], woff[:, 0:2], gl[:, sl],
                                             start=True, stop=True)
                        th = WK.tile([2, N], F32, tag="th", name="th")
                        nc.scalar.activation(th[:], z2[:], AF.Tanh)
                        nc.vector.scalar_tensor_tensor(syx[g][:], th[:], 0.96875,
                                                       tyxb[:], op0=OP.mult, op1=OP.add)
                        dump("syx", g, syx[g][:])

                # ---------- phase 3+4: tent weights, resample S, x_sampled^T ----------
                for g in range(2):
                    with tc.tile_pool(name="spool", bufs=8) as SP:
                      with tc.tile_pool(name="pst", bufs=1, space=PSUM) as PST:
                        sb2 = PST.tile([64, N], F32, tag="sbc", name="sbc")
                        for ic in range(2):
                            sl = slice(ic * 512, (ic + 1) * 512)
                            nc.tensor.matmul(sb2[:, sl], sel2[:], syx[g][:, sl],
                                             start=True, stop=True)
                        dab = WKB.tile([64, N], F32, tag="dab", name="dab")
                        nc.scalar.activation(dab[:], sb2[:], AF.Abs, bias=npy64[:, 0:1], scale=1.0)
                        ts2 = WK.tile([64, N], BF16, tag="tsm2", name="tsm2")
                        nc.scalar.activation(ts2[:], dab[:], AF.Relu, bias=1.0, scale=-1.0)
                        tsm = [ts2[0:32, :], ts2[32:64, :]]
                        dump("ts2", g, ts2[:], cast=[64, N])
                      with tc.tile_pool(name="psr", bufs=1, space=PSUM) as PSR, \
                           tc.tile_pool(name="psx", bufs=2, space=PSUM) as PSX:
                        txr = PSR.tile([128, N], F32, tag="txr", name="txr")
                        for ic in range(2):
                            sl = slice(ic * 512, (ic + 1) * 512)
                            nc.tensor.matmul(txr[:, sl], eptx[32:64, 1024:1152],
                                             tsm[1][:, sl], start=True, stop=True,
                                             tile_position=(32, 0))
                        txs = WKB.tile([128, N], BF16, tag="txs", name="txs")
                        nc.vector.tensor_copy(txs[:], txr[:])
                        Sg = []
                        for pt in range(8):
                            tyr = PSR.tile([128, N], F32, tag="tyr", name="tyr", bufs=2)
                            for ic in range(2):
                                sl = slice(ic * 512, (ic + 1) * 512)
                                nc.tensor.matmul(tyr[:, sl], eptx[0:32, pt * 128:(pt + 1) * 128],
                                                 tsm[0][:, sl], start=True, stop=True)
                            St = SP.tile([128, N], BF16, tag="S")
                            nc.vector.tensor_tensor(St[:], tyr[:], txs[:], op=OP.mult)
                            if g == 0 and pt == 0:
                                dump("S0", None, St[:], cast=[128, N])
                            Sg.append(St)
                        for jc in range(2):
                            ps = PSX.tile([128, 512], F32, tag="psxs", name="psxs")
                            for pt in range(8):
                                nc.tensor.matmul(ps[:], qti[pt][:, g * 128:(g + 1) * 128],
                                                 Sg[pt][:, jc * 512:(jc + 1) * 512],
                                                 start=pt == 0, stop=pt == 7)
                            nc.vector.tensor_copy(xss[g][:, jc * 512:(jc + 1) * 512], ps[:])
                    dump("xsT", g, xss[g][:], cast=[128, N])

                # ---------- phase 5: k^T, v ----------
                with tc.tile_pool(name="ps5", bufs=4, space=PSUM) as PS5:
                    for dt2 in range(2):
                        for jc in range(2):
                            ps = PS5.tile([128, 512], F32, tag="psk")
                            for ct in range(2):
                                nc.tensor.matmul(ps[:], Wks[ct][:, dt2 * 128:(dt2 + 1) * 128],
                                                 xss[ct][:, jc * 512:(jc + 1) * 512],
                                                 start=ct == 0, stop=ct == 1)
                            nc.vector.tensor_copy(kTs[dt2][:, jc * 512:(jc + 1) * 512], ps[:])
                    for dt2 in range(2):
                        dump("kT", dt2, kTs[dt2][:], cast=[128, N])
                    for jt in range(8):
                        ps = PS5.tile([128, 256], F32, tag="psv")
                        for ct in range(2):
                            nc.tensor.matmul(ps[:], xss[ct][:, jt * 128:(jt + 1) * 128],
                                             Wvs[ct][:], start=ct == 0, stop=ct == 1)
                        v3 = v1s[jt][:, :].rearrange("p (h c) -> p h c", c=33)
                        nc.vector.tensor_copy(v3[:, :, 0:32],
                                              ps[:, :].rearrange("p (h c) -> p h c", c=32))
                        nc.gpsimd.memset(v3[:, :, 32:33], 1.0)

                # ---------- phase 6: attention with bias ----------
                for g in range(2):
                    for p2 in range(2):
                        cs = (2 * p2, 2 * p2 + 1)
                        with tc.tile_pool(name="pssim", bufs=2, space=PSUM) as PSS, \
                             tc.tile_pool(name="psout", bufs=4, space=PSUM) as PSO:
                            ot = {(c, ic): PSO.tile([33, 512], F32, tag="ot", name=f"ot{c}_{ic}")
                                  for c in cs for ic in range(2)}
                            for jt in range(8):
                                for c in cs:
                                    h = g * 4 + c
                                    eb = EBP.tile([128, N], BF16, tag="eb")
                                    nc.sync.dma_start(eb[:], ebx_d.ap()[h][jt])
                                    sim = PSS.tile([128, 1024], F32, tag="sim")
                                    for ic in range(2):
                                        nc.tensor.matmul(
                                            sim[:, ic * 512:(ic + 1) * 512],
                                            kTs[g][c * 32:(c + 1) * 32, jt * 128:(jt + 1) * 128],
                                            qTs[g][c * 32:(c + 1) * 32, ic * 512:(ic + 1) * 512],
                                            start=True, stop=True,
                                            tile_position=(c * 32, 0))
                                    es = ESP.tile([128, N], BF16, tag="es")
                                    nc.scalar.activation(es[:], sim[:], AF.Exp)
                                    pt_ = ESP.tile([128, N], BF16, tag="pt")
                                    eng = nc.gpsimd if jt in (2, 5) else nc.vector
                                    eng.tensor_tensor(pt_[:], es[:], eb[:], op=OP.mult)
                                    if h == 0 and jt == 0:
                                        dump("es0", None, pt_[:], cast=[128, N])
                                        dump("eb0", None, eb[:], cast=[128, N])
                                    for ic in range(2):
                                        nc.tensor.matmul(
                                            ot[(c, ic)][:],
                                            v1s[jt][:, h * 33:(h + 1) * 33],
                                            pt_[:, ic * 512:(ic + 1) * 512],
                                            start=jt == 0, stop=jt == 7)
                            # denominators -> reciprocal rows, copy outT to SBUF
                            rrow = {}
                            ots = {}
                            for cl, c in enumerate(cs):
                                rw = RR2P.tile([1, N], F32, tag=f"rr{cl}", name="rw")
                                rwt = WK.tile([1, N], F32, tag="rwt", name="rwt")
                                for ic in range(2):
                                    sl = slice(ic * 512, (ic + 1) * 512)
                                    nc.scalar.activation(rwt[:, sl],
                                                         ot[(c, ic)][32:33, :], AF.Copy)
                                    o_s = OTS.tile([32, 512], F32, tag="otsb", name="otsb")
                                    nc.vector.tensor_copy(o_s[:], ot[(c, ic)][0:32, :])
                                    ots[(c, ic)] = o_s
                                nc.vector.reciprocal_approx_fast(rw[:], rwt[:])
                                rrow[c] = rw
                        with tc.tile_pool(name="psr2", bufs=2, space=PSUM) as PSR2:
                            for cl, c in enumerate(cs):
                                R1 = PSR2.tile([32, N], F32, tag="R1", name="R1")
                                for ic in range(2):
                                    sl = slice(ic * 512, (ic + 1) * 512)
                                    nc.tensor.matmul(R1[:, sl], o32r[:], rrow[c][:, sl],
                                                     start=True, stop=True)
                                for ic in range(2):
                                    sl = slice(ic * 512, (ic + 1) * 512)
                                    nc.vector.tensor_tensor(
                                        nss[g][c * 32:(c + 1) * 32, sl],
                                        ots[(c, ic)][:],
                                        R1[:, sl], op=OP.mult)

                for g in range(2):
                    dump("ns", g, nss[g][:], cast=[128, N])

                # ---------- phase 7: output projection ----------
                with tc.tile_pool(name="ps7", bufs=4, space=PSUM) as PS7:
                    for it in range(8):
                        ps = PS7.tile([128, DIM], F32, tag="psy")
                        for g in range(2):
                            nc.tensor.matmul(ps[:], nss[g][:, it * 128:(it + 1) * 128],
                                             Wos[g][:], start=g == 0, stop=g == 1)
                        yo = OTS.tile([128, DIM], F32, tag="yo", name="yo")
                        nc.vector.tensor_tensor(yo[:], ps[:], boutb[:], op=OP.add)
                        nc.sync.dma_start(y_d.ap()[it], yo[:])

            if reps == 1:
                phases()
            else:
                E = mybir.EngineType
                with tc.For_i(0, reps, 1,
                              hint_engines=(E.PE, E.DVE, E.Activation, E.Pool, E.SP)):
                    phases()
    return nc


def _make_eptx():
    bf = ml_dtypes.bfloat16
    ept = np.stack([(np.arange(32)[:, None] == (pt * 4 + np.arange(128)[None, :] // 32))
                    for pt in range(8)]).astype(bf).transpose(1, 0, 2).reshape(32, 8 * 128)
    epx = (np.arange(32)[:, None] == (np.arange(128)[None, :] % 32)).astype(bf)
    comb = np.zeros((64, 1152), bf)
    comb[0:32, 0:1024] = ept
    comb[32:64, 1024:1152] = epx
    return comb


def _tent_mat():
    d = np.arange(63)[:, None]
    p = np.arange(63)[None, :]
    pos = 31.0 + 0.96875 * (d - 31)
    return np.maximum(0.0, 1.0 - np.abs(pos - p)).astype(np.float32)


def prep_consts(Wq, Wkv, conv_w, conv_b, ln_g, ln_b, Woff, rpe, Wout, bout):
    bf = ml_dtypes.bfloat16
    scale = np.float32(32 ** -0.5)
    WY = _tent_mat()
    T = np.einsum('ap,hpq,bq->hab', WY, rpe.astype(np.float32), WY)
    expT = np.exp(T).astype(np.float32)  # [8, 63, 63]
    # expanded bias table: ebx[h, jt, p, i] = expT[h, 31+iy-jy, 31+ix-jx]
    # with j = jt*128+p, (jy,jx) = divmod(j,32), (iy,ix) = divmod(i,32)
    j = np.arange(N)
    i = np.arange(N)
    jy, jx = j // 32, j % 32
    iy, ix = i // 32, i % 32
    A = 31 + iy[None, :] - jy[:, None]   # [N, N] y-displacement index
    B = 31 + ix[None, :] - jx[:, None]   # [N, N] x-displacement index
    ebx = np.empty((8, N, N), bf)
    for h in range(8):
        ebx[h] = expT[h][A, B].astype(bf)
    ebx = ebx.reshape(8, 8, 128, N)
    jyf = (np.arange(N) // 32).astype(np.float32)
    jxf = (np.arange(N) % 32).astype(np.float32)
    return dict(
        Wq=Wq.astype(bf).reshape(2, 128, DIM),
        Wk=(Wkv[:, :DIM] * scale).astype(bf).reshape(2, 128, DIM),
        Wv=Wkv[:, DIM:].astype(bf).reshape(2, 128, DIM),
        Wo=Wout.astype(bf).reshape(2, 128, DIM),
        Woff=Woff.astype(np.float32),
        diagw=(np.eye(128, dtype=np.float32)[None, :, :]
               * conv_w.reshape(128, 25).astype(np.float32).T[:, :, None]
               ).transpose(1, 0, 2).reshape(128, 25 * 128).astype(bf),
        convb=conv_b.reshape(128, 1).astype(np.float32),
        cbmean=np.array([[conv_b.mean()]], np.float32),
        lng=ln_g.reshape(128, 1).astype(np.float32),
        lnb=ln_b.reshape(128, 1).astype(np.float32),
        npy64=np.concatenate([-np.arange(32, dtype=np.float32)] * 2).reshape(64, 1),
        Eptx=_make_eptx(),
        tyxb=np.stack([(jyf + 0.5) * 0.96875, (jxf + 0.5) * 0.96875]).astype(np.float32),
        inv128=np.full((128, 1), 1.0 / 128, np.float32),
        ones32r=np.ones((1, 32), np.float32),
        ones128r=np.ones((1, 128), np.float32),
        sel2=(np.arange(2)[:, None] == (np.arange(64)[None, :] // 32)).astype(np.float32),
        ebx=ebx,
        boutb=np.tile(bout.reshape(1, DIM), (128, 1)).astype(np.float32),
    )


def prep_x(x):
    """x [8, N, DIM] f32 -> concat xT [8*2, 128, N] bf16 (sharded by core)."""
    bf = ml_dtypes.bfloat16
    # [B, N, DIM] -> [B, DIM, N] -> [B, 2, 128, N] -> [B*2, 128, N]
    return np.ascontiguousarray(x.transpose(0, 2, 1)).astype(bf).reshape(8 * 2, 128, N)


_CACHE = {}


def _weights_key(ws):
    hsh = hashlib.sha1()
    for w in ws:
        hsh.update(np.ascontiguousarray(w).tobytes())
    return hsh.hexdigest()


def make_sharded(consts, reps=1):
    """Build + compile the Bass module and wrap it in an 8-core sharded jit."""
    import jax
    from concourse.bass2jax import (_bass_exec_p, install_neuronx_cc_hook,
                                    partition_id_tensor)
    from jax.sharding import Mesh, PartitionSpec
    from jax.experimental.shard_map import shard_map

    install_neuronx_cc_hook()
    nc = bacc.Bacc("TRN2", target_bir_lowering=False, debug=False)
    build(nc, consts, reps=reps)
    nc.compile()
    n_cores = 8
    in_names, out_names, out_avals, zero_outs = [], [], [], []
    for alloc in nc.m.functions[0].allocations:
        if not isinstance(alloc, mybir.MemoryLocationSet):
            continue
        nm = alloc.memorylocations[0].name
        if alloc.kind == "ExternalInput":
            if nc.partition_id_tensor is None or nm != nc.partition_id_tensor.name:
                in_names.append(nm)
        elif alloc.kind == "ExternalOutput":
            out_names.append(nm)
            shape = tuple(alloc.tensor_shape)
            dtype = mybir.dt.np(alloc.dtype)
            out_avals.append(jax.core.ShapedArray(shape, dtype))
            zero_outs.append(np.zeros((n_cores * shape[0], *shape[1:]), dtype))
    assert set(in_names) == {"xT", "ebx"}, in_names
    all_names = in_names + out_names
    if nc.partition_id_tensor is not None:
        all_names.append(nc.partition_id_tensor.name)

    def _body(*args):
        operands = list(args)
        if nc.partition_id_tensor is not None:
            operands.append(partition_id_tensor())
        return tuple(_bass_exec_p.bind(
            *operands, out_avals=tuple(out_avals), in_names=tuple(all_names),
            out_names=tuple(out_names), lowering_input_output_aliases=(),
            sim_require_finite=True, sim_require_nnan=True, nc=nc))

    devices = jax.devices()[:n_cores]
    mesh = Mesh(np.asarray(devices), ("core",))
    # ebx is identical on every core: pass it replicated (no 8x concat)
    in_sp = tuple(PartitionSpec() if nm == "ebx" else PartitionSpec("core")
                  for nm in in_names)
    sharded = jax.jit(
        shard_map(_body, mesh=mesh,
                  in_specs=in_sp + (PartitionSpec("core"),) * len(out_names),
                  out_specs=(PartitionSpec("core"),) * len(out_names),
                  check_rep=False),
        keep_unused=True)
    return sharded, in_names, out_names, zero_outs


def kernel(x, Wq, Wkv, conv_w, conv_b, ln_g, ln_b, Woff, rpe, Wout, bout):
    import jax
    from jax.sharding import Mesh, PartitionSpec, NamedSharding
    ws = (Wq, Wkv, conv_w, conv_b, ln_g, ln_b, Woff, rpe, Wout, bout)
    wkey = _weights_key(ws)
    if _CACHE.get('wkey') != wkey:
        consts = prep_consts(*[np.asarray(w) for w in ws])
        _CACHE['sharded'] = make_sharded(consts)
        mesh = Mesh(np.asarray(jax.devices()[:8]), ("core",))
        sh = NamedSharding(mesh, PartitionSpec())
        _CACHE['ebx_dev'] = jax.device_put(consts['ebx'], sh)
        _CACHE['wkey'] = wkey
    sharded, in_names, out_names, zero_outs = _CACHE['sharded']
    xT = prep_x(np.asarray(x))
    args = [xT if nm == 'xT' else _CACHE['ebx_dev'] for nm in in_names]
    outs = sharded(*args, *zero_outs)
    y = np.asarray(outs[out_names.index("y")])
    return y.reshape(8, 8, 128, DIM).reshape(8, N, DIM).astype(np.float32)


if __name__ == '__main__':
    rng = np.random.default_rng(0)
    ins = dict(
        x=rng.standard_normal((8, N, DIM), np.float32),
        Wq=rng.standard_normal((DIM, DIM), np.float32) * 0.02,
        Wkv=rng.standard_normal((DIM, 2 * DIM), np.float32) * 0.02,
        conv_w=rng.standard_normal((128, 1, 5, 5), np.float32) * 0.02,
        conv_b=np.zeros(128, np.float32),
        ln_g=np.ones(128, np.float32),
        ln_b=np.zeros(128, np.float32),
        Woff=rng.standard_normal((128, 2), np.float32) * 0.02,
        rpe=rng.standard_normal((8, 63, 63), np.float32) * 0.01,
        Wout=rng.standard_normal((DIM, DIM), np.float32) * 0.02,
        bout=np.zeros(DIM, np.float32),
    )
    y = kernel(**ins)
    print('out', y.shape, y.dtype, float(np.abs(y).max()))


# revision 3
# speedup vs baseline: 1.0642x; 1.0642x over previous
"""DeformableAttention on 8 Trainium2 NeuronCores via a hand-written Bass/Tile kernel.

Data-parallel over batch: one batch item per core, identical SPMD program.
All weight-derived tensors are baked into the NEFF as Const (inline_tensor);
the only runtime input is xT, the only output y.

v2 changes vs baseline:
- relative-position bias table host-expanded to [8h, 8jt, 128, 1024] bf16 so
  the per-(head, j-tile) load is one fully contiguous 256KB DMA (the old
  Toeplitz gather moved 16MB/core in 64B chunks, far below DMA line rate).
- conv taps and attention bias multiplies split across DVE and GPSIMD(Pool).
- LN rsqrt and softmax denominators use Rsqrt/Reciprocal activation tables
  (1 op) instead of Sqrt/Ln/Exp chains (3 ops).
- offset tanh + tent ops batched across the y/x axes ([2,N] / [64,N] tiles).
- optional `reps` hardware loop (tc.For_i) repeating the whole body for
  device-time measurement with dispatch overhead cancelled.
"""
import hashlib
import numpy as np
import ml_dtypes
import concourse.bass as bass
import concourse.bacc as bacc
import concourse.mybir as mybir
import concourse.tile as tile

F32 = mybir.dt.float32
BF16 = mybir.dt.bfloat16
AF = mybir.ActivationFunctionType
OP = mybir.AluOpType
PSUM = bass.MemorySpace.PSUM

H = 32
N = 1024
DIM = 256
EPS = 1e-5


def build(nc, consts, reps=1, dbg=False):
    xT_d = nc.dram_tensor("xT", [2, 128, N], BF16, kind="ExternalInput")
    y_d = nc.dram_tensor("y", [8, 128, DIM], F32, kind="ExternalOutput")
    dbg_d = {}
    if dbg:
        for nm, shp in [("qT", [2, 128, N]), ("acc", [2, 128, N]),
                        ("syx", [2, 2, N]), ("ts2", [2, 64, N]),
                        ("S0", [128, N]), ("xsT", [2, 128, N]),
                        ("kT", [2, 128, N]), ("es0", [128, N]),
                        ("eb0", [128, N]), ("ns", [2, 128, N])]:
            dbg_d[nm] = nc.dram_tensor("dbg_" + nm, shp, F32, kind="ExternalOutput")

    def cin(name):
        return nc.inline_tensor(np.ascontiguousarray(consts[name]), name=name)

    Wq_d = cin("Wq")      # [2,128,DIM] bf16
    Wk_d = cin("Wk")
    Wv_d = cin("Wv")
    Wo_d = cin("Wo")
    woff_d = cin("Woff")  # [128,2] f32
    diagw_d = cin("diagw")  # [128, 25*128] bf16: tap t cols t*128:(t+1)*128 = diag(conv_w[:,t])
    convb_d = cin("convb")  # [128,1] f32
    cbm_d = cin("cbmean")   # [1,1] f32
    lng_d = cin("lng")
    lnb_d = cin("lnb")
    npy64_d = cin("npy64")  # [64,1] f32
    eptx_d = cin("Eptx")    # [64, 1152] bf16: rows 0:32 Ept, rows 32:64 cols 1024:1152 Epx
    tyxb_d = cin("tyxb")    # [2,N] f32
    o128_d = cin("inv128")  # [128,1] f32
    o32r_d = cin("ones32r")  # [1,32] f32
    o128r_d = cin("ones128r")  # [1,128] f32
    sel2_d = cin("sel2")    # [2,64] f32
    # expanded bias table: runtime input (device-resident, uploaded once),
    # NOT inline — a 16MB Const in the NEFF costs ~120ms per dispatch through
    # the axon relay (per-call cost scales with executable size).
    ebx_d = nc.dram_tensor("ebx", [8, 8, 128, N], BF16, kind="ExternalInput")
    bout_d = cin("boutb")   # [128,DIM] f32

    with tile.TileContext(nc) as tc:
        import contextlib
        ctx = contextlib.ExitStack()
        with ctx:
            P = ctx.enter_context(tc.tile_pool(name="persist", bufs=1))
            WKB = ctx.enter_context(tc.tile_pool(name="workbig", bufs=1))
            WK = ctx.enter_context(tc.tile_pool(name="work", bufs=1))
            RR2P = ctx.enter_context(tc.tile_pool(name="rr2pool", bufs=2))
            OTS = ctx.enter_context(tc.tile_pool(name="otspool", bufs=4))
            EBP = ctx.enter_context(tc.tile_pool(name="ebpool", bufs=6))
            ESP = ctx.enter_context(tc.tile_pool(name="espool", bufs=4))

            def load(d, shape, dt, tag, sl=None):
                t = P.tile(shape, dt, tag=tag, name=tag)
                nc.sync.dma_start(t[:], d.ap() if sl is None else d.ap()[sl])
                return t

            xTs = [load(xT_d, [128, N], BF16, f"xT{k}", k) for k in range(2)]
            Wqs = [load(Wq_d, [128, DIM], BF16, f"Wq{k}", k) for k in range(2)]
            Wks = [load(Wk_d, [128, DIM], BF16, f"Wk{k}", k) for k in range(2)]
            Wvs = [load(Wv_d, [128, DIM], BF16, f"Wv{k}", k) for k in range(2)]
            Wos = [load(Wo_d, [128, DIM], BF16, f"Wo{k}", k) for k in range(2)]
            woff = load(woff_d, [128, 2], F32, "woff")
            diagw = load(diagw_d, [128, 25 * 128], BF16, "diagw")
            convb = load(convb_d, [128, 1], F32, "convb")
            cbm = load(cbm_d, [1, 1], F32, "cbm")
            lng = load(lng_d, [128, 1], F32, "lng")
            lnb = load(lnb_d, [128, 1], F32, "lnb")
            npy64 = load(npy64_d, [64, 1], F32, "npy64")
            eptx = load(eptx_d, [64, 1152], BF16, "eptx")
            tyxb = load(tyxb_d, [2, N], F32, "tyxb")
            inv128 = load(o128_d, [128, 1], F32, "inv128")
            o32r = load(o32r_d, [1, 32], F32, "o32r")
            o128r = load(o128r_d, [1, 128], F32, "o128r")
            sel2 = load(sel2_d, [2, 64], F32, "sel2")
            boutb = load(bout_d, [128, DIM], F32, "boutb")

            qTs = [P.tile([128, N], BF16, tag=f"qT{k}", name=f"qT{k}") for k in range(2)]
            qti = [P.tile([128, DIM], BF16, tag=f"q{k}", name=f"q{k}") for k in range(8)]
            xss = [P.tile([128, N], BF16, tag=f"xs{k}", name=f"xs{k}") for k in range(2)]
            kTs = [P.tile([128, N], BF16, tag=f"kT{k}", name=f"kT{k}") for k in range(2)]
            v1s = [P.tile([128, 264], BF16, tag=f"v1{k}", name=f"v1{k}") for k in range(8)]
            nss = [P.tile([128, N], BF16, tag=f"ns{k}", name=f"ns{k}") for k in range(2)]
            syx = [P.tile([2, N], F32, tag=f"syx{k}", name=f"syx{k}") for k in range(2)]
            # zero-padded 36x36 q feature maps for the PE depthwise conv;
            # borders zeroed once here (they are never written inside the body).
            # 1332 = 37*36: one spare row so every tap's flat 1152-slice stays
            # in bounds (max read offset 148+1152=1300).
            qps = [P.tile([128, 1332], BF16, tag=f"qp{k}", name=f"qp{k}") for k in range(2)]
            for g in range(2):
                nc.gpsimd.memset(qps[g][:], 0.0)

            def dump(nm, sl, src, cast=None):
                if not dbg:
                    return
                if cast is not None:
                    t = WKB.tile(cast, F32, tag="dbgbig", name="dbgbig")
                    nc.vector.tensor_copy(t[:], src)
                    src = t[:]
                dst = dbg_d[nm].ap() if sl is None else dbg_d[nm].ap()[sl]
                nc.sync.dma_start(dst, src)

            def phases():
                # ---------- phase 1: qT and q ----------
                with tc.tile_pool(name="ps1", bufs=4, space=PSUM) as PS1:
                    for dt in range(2):
                        for ic in range(2):
                            ps = PS1.tile([128, 512], F32, tag="psq")
                            for kt in range(2):
                                nc.tensor.matmul(ps[:], Wqs[kt][:, dt * 128:(dt + 1) * 128],
                                                 xTs[kt][:, ic * 512:(ic + 1) * 512],
                                                 start=kt == 0, stop=kt == 1)
                            if ic == 0:
                                nc.vector.tensor_copy(qTs[dt][:, ic * 512:(ic + 1) * 512], ps[:])
                            else:
                                nc.scalar.activation(qTs[dt][:, ic * 512:(ic + 1) * 512], ps[:], AF.Copy)
                    for it in range(8):
                        ps = PS1.tile([128, 256], F32, tag="psq2")
                        for kt in range(2):
                            nc.tensor.matmul(ps[:], xTs[kt][:, it * 128:(it + 1) * 128],
                                             Wqs[kt][:], start=kt == 0, stop=kt == 1)
                        if it % 2 == 0:
                            nc.vector.tensor_copy(qti[it][:], ps[:])
                        else:
                            nc.scalar.activation(qti[it][:], ps[:], AF.Copy)
                for dt in range(2):
                    dump("qT", dt, qTs[dt][:], cast=[128, N])

                # ---------- phase 2: conv offset -> LN -> GELU -> offsets ----------
                for g in range(2):
                    # copy q feature map into the interior of the padded tile
                    qg3 = qTs[g][:, :].rearrange("p (y x) -> p y x", y=32)
                    qp3 = qps[g][:, 0:1296].rearrange("p (y x) -> p y x", y=36)
                    nc.vector.tensor_copy(qp3[:, 2:34, 2:34], qg3[:, :, :])
                    # depthwise 5x5 conv as 25 accumulating diag-matmuls on PE.
                    # output is 36-wide padded rows so each tap's rhs is one
                    # flat contiguous slice of the padded input.
                    taps = [(dy, dx) for dy in range(-2, 3) for dx in range(-2, 3)]
                    with tc.tile_pool(name="psconv", bufs=1, space=PSUM) as PSC:
                        accp = PSC.tile([128, 1152], F32, tag="accp", name="accp")
                        for ti, (dy, dx) in enumerate(taps):
                            tap = (dy + 2) * 5 + (dx + 2)
                            st = (dy + 2) * 36 + (dx + 2)
                            for cs_, ce_ in ((0, 512), (512, 1024), (1024, 1152)):
                                nc.tensor.matmul(
                                    accp[:, cs_:ce_],
                                    diagw[:, tap * 128:(tap + 1) * 128],
                                    qps[g][:, st + cs_:st + ce_],
                                    start=ti == 0, stop=ti == 24)
                        acc = WKB.tile([128, N], F32, tag="acc" + str(g), name="acc")
                        nc.vector.tensor_copy(
                            acc[:, :].rearrange("p (y x) -> p y x", y=32),
                            accp[:, :].rearrange("p (y x) -> p y x", y=32)[:, :, 0:32])
                        dump("acc", g, acc[:])
                        sq = WKB.tile([128, N], F32, tag="sq", name="sq")
                        nc.scalar.activation(sq[:], acc[:], AF.Square, bias=convb[:, 0:1], scale=1.0)
                    with tc.tile_pool(name="psln", bufs=1, space=PSUM) as PSL:
                        mu0 = PSL.tile([1, N], F32, tag="mu0")
                        msq = PSL.tile([1, N], F32, tag="msq")
                        for ic in range(2):
                            sl = slice(ic * 512, (ic + 1) * 512)
                            nc.tensor.matmul(mu0[:, sl], inv128[:], acc[:, sl], start=True, stop=True)
                            nc.tensor.matmul(msq[:, sl], inv128[:], sq[:, sl], start=True, stop=True)
                        mu = WK.tile([1, N], F32, tag="mu", name="mu")
                        nc.vector.tensor_scalar(mu[:], mu0[:], cbm[:, 0:1], None, op0=OP.add)
                        r0 = WK.tile([1, N], F32, tag="r0", name="r0")
                        r1 = WK.tile([1, N], F32, tag="r1", name="r1")
                        nc.vector.tensor_tensor(r0[:], mu[:], mu[:], op=OP.mult)
                        nc.vector.scalar_tensor_tensor(r1[:], msq[:], EPS, r0[:],
                                                       op0=OP.add, op1=OP.subtract)
                        ri = WK.tile([1, N], F32, tag="ri", name="ri")
                        nc.vector.reciprocal_approx_fast(ri[:], r1[:])
                        rr = r0
                        nc.scalar.activation(rr[:], ri[:], AF.Sqrt)
                        mur = r1
                        nc.vector.tensor_tensor(mur[:], mu[:], rr[:], op=OP.mult)
                        Rb = PSL.tile([128, N], F32, tag="Rb")
                        Mb = PSL.tile([128, N], F32, tag="Mb")
                        for ic in range(2):
                            sl = slice(ic * 512, (ic + 1) * 512)
                            nc.tensor.matmul(Rb[:, sl], o128r[:], rr[:, sl], start=True, stop=True)
                            nc.tensor.matmul(Mb[:, sl], o128r[:], mur[:, sl], start=True, stop=True)
                        t1 = WKB.tile([128, N], F32, tag="t1", name="t1")
                        nc.vector.scalar_tensor_tensor(t1[:], acc[:], convb[:, 0:1], Rb[:],
                                                       op0=OP.add, op1=OP.mult)
                        on = WKB.tile([128, N], F32, tag="on", name="on")
                        nc.vector.scalar_tensor_tensor(on[:], t1[:], 0.0, Mb[:],
                                                       op0=OP.add, op1=OP.subtract)
                    u = WKB.tile([128, N], F32, tag="u", name="u")
                    nc.vector.tensor_scalar(u[:], on[:], lng[:, 0:1], lnb[:, 0:1],
                                            op0=OP.mult, op1=OP.add)
                    sg = WKB.tile([128, N], F32, tag="sg", name="sg")
                    nc.scalar.activation(sg[:], u[:], AF.Sigmoid, scale=1.702)
                    gl = WKB.tile([128, N], F32, tag="gl", name="gl")
                    nc.gpsimd.tensor_tensor(gl[:], u[:], sg[:], op=OP.mult)
                    with tc.tile_pool(name="psz", bufs=2, space=PSUM) as PSZ:
                        z2 = PSZ.tile([2, N], F32, tag="z2", name="z2")
                        for ic in range(2):
                            sl = slice(ic * 512, (ic + 1) * 512)
                            nc.tensor.matmul(z2[:, sl], woff[:, 0:2], gl[:, sl],
                                             start=True, stop=True)
                        th = WK.tile([2, N], F32, tag="th", name="th")
                        nc.scalar.activation(th[:], z2[:], AF.Tanh)
                        nc.vector.scalar_tensor_tensor(syx[g][:], th[:], 0.96875,
                                                       tyxb[:], op0=OP.mult, op1=OP.add)
                        dump("syx", g, syx[g][:])

                # ---------- phase 3+4: tent weights, resample S, x_sampled^T ----------
                for g in range(2):
                    with tc.tile_pool(name="spool", bufs=8) as SP:
                      with tc.tile_pool(name="pst", bufs=1, space=PSUM) as PST:
                        sb2 = PST.tile([64, N], F32, tag="sbc", name="sbc")
                        for ic in range(2):
                            sl = slice(ic * 512, (ic + 1) * 512)
                            nc.tensor.matmul(sb2[:, sl], sel2[:], syx[g][:, sl],
                                             start=True, stop=True)
                        dab = WKB.tile([64, N], F32, tag="dab", name="dab")
                        nc.scalar.activation(dab[:], sb2[:], AF.Abs, bias=npy64[:, 0:1], scale=1.0)
                        ts2 = WK.tile([64, N], BF16, tag="tsm2", name="tsm2")
                        nc.scalar.activation(ts2[:], dab[:], AF.Relu, bias=1.0, scale=-1.0)
                        tsm = [ts2[0:32, :], ts2[32:64, :]]
                        dump("ts2", g, ts2[:], cast=[64, N])
                      with tc.tile_pool(name="psr", bufs=1, space=PSUM) as PSR, \
                           tc.tile_pool(name="psx", bufs=2, space=PSUM) as PSX:
                        txr = PSR.tile([128, N], F32, tag="txr", name="txr")
                        for ic in range(2):
                            sl = slice(ic * 512, (ic + 1) * 512)
                            nc.tensor.matmul(txr[:, sl], eptx[32:64, 1024:1152],
                                             tsm[1][:, sl], start=True, stop=True,
                                             tile_position=(32, 0))
                        txs = WKB.tile([128, N], BF16, tag="txs", name="txs")
                        nc.vector.tensor_copy(txs[:], txr[:])
                        Sg = []
                        for pt in range(8):
                            tyr = PSR.tile([128, N], F32, tag="tyr", name="tyr", bufs=2)
                            for ic in range(2):
                                sl = slice(ic * 512, (ic + 1) * 512)
                                nc.tensor.matmul(tyr[:, sl], eptx[0:32, pt * 128:(pt + 1) * 128],
                                                 tsm[0][:, sl], start=True, stop=True)
                            St = SP.tile([128, N], BF16, tag="S")
                            nc.vector.tensor_tensor(St[:], tyr[:], txs[:], op=OP.mult)
                            if g == 0 and pt == 0:
                                dump("S0", None, St[:], cast=[128, N])
                            Sg.append(St)
                        for jc in range(2):
                            ps = PSX.tile([128, 512], F32, tag="psxs", name="psxs")
                            for pt in range(8):
                                nc.tensor.matmul(ps[:], qti[pt][:, g * 128:(g + 1) * 128],
                                                 Sg[pt][:, jc * 512:(jc + 1) * 512],
                                                 start=pt == 0, stop=pt == 7)
                            nc.vector.tensor_copy(xss[g][:, jc * 512:(jc + 1) * 512], ps[:])
                    dump("xsT", g, xss[g][:], cast=[128, N])

                # ---------- phase 5: k^T, v ----------
                with tc.tile_pool(name="ps5", bufs=4, space=PSUM) as PS5:
                    for dt2 in range(2):
                        for jc in range(2):
                            ps = PS5.tile([128, 512], F32, tag="psk")
                            for ct in range(2):
                                nc.tensor.matmul(ps[:], Wks[ct][:, dt2 * 128:(dt2 + 1) * 128],
                                                 xss[ct][:, jc * 512:(jc + 1) * 512],
                                                 start=ct == 0, stop=ct == 1)
                            if jc == 0:
                                nc.vector.tensor_copy(kTs[dt2][:, jc * 512:(jc + 1) * 512], ps[:])
                            else:
                                nc.scalar.activation(kTs[dt2][:, jc * 512:(jc + 1) * 512], ps[:], AF.Copy)
                    for dt2 in range(2):
                        dump("kT", dt2, kTs[dt2][:], cast=[128, N])
                    for jt in range(8):
                        ps = PS5.tile([128, 256], F32, tag="psv")
                        for ct in range(2):
                            nc.tensor.matmul(ps[:], xss[ct][:, jt * 128:(jt + 1) * 128],
                                             Wvs[ct][:], start=ct == 0, stop=ct == 1)
                        v3 = v1s[jt][:, :].rearrange("p (h c) -> p h c", c=33)
                        if jt % 2 == 0:
                            nc.vector.tensor_copy(v3[:, :, 0:32],
                                                  ps[:, :].rearrange("p (h c) -> p h c", c=32))
                        else:
                            nc.scalar.activation(v3[:, :, 0:32],
                                                 ps[:, :].rearrange("p (h c) -> p h c", c=32),
                                                 AF.Copy)
                        nc.gpsimd.memset(v3[:, :, 32:33], 1.0)

                # ---------- phase 6: attention with bias ----------
                for g in range(2):
                    for p2 in range(2):
                        cs = (2 * p2, 2 * p2 + 1)
                        with tc.tile_pool(name="pssim", bufs=2, space=PSUM) as PSS, \
                             tc.tile_pool(name="psout", bufs=4, space=PSUM) as PSO:
                            ot = {(c, ic): PSO.tile([33, 512], F32, tag="ot", name=f"ot{c}_{ic}")
                                  for c in cs for ic in range(2)}
                            for jt in range(8):
                                for c in cs:
                                    h = g * 4 + c
                                    eb = EBP.tile([128, N], BF16, tag="eb")
                                    nc.sync.dma_start(eb[:], ebx_d.ap()[h][jt])
                                    sim = PSS.tile([128, 1024], F32, tag="sim")
                                    for ic in range(2):
                                        nc.tensor.matmul(
                                            sim[:, ic * 512:(ic + 1) * 512],
                                            kTs[g][c * 32:(c + 1) * 32, jt * 128:(jt + 1) * 128],
                                            qTs[g][c * 32:(c + 1) * 32, ic * 512:(ic + 1) * 512],
                                            start=True, stop=True,
                                            tile_position=(c * 32, 0))
                                    es = ESP.tile([128, N], BF16, tag="es")
                                    nc.scalar.activation(es[:], sim[:], AF.Exp)
                                    pt_ = ESP.tile([128, N], BF16, tag="pt")
                                    eng = nc.gpsimd if jt in (1, 4, 7) else nc.vector
                                    eng.tensor_tensor(pt_[:], es[:], eb[:], op=OP.mult)
                                    if h == 0 and jt == 0:
                                        dump("es0", None, pt_[:], cast=[128, N])
                                        dump("eb0", None, eb[:], cast=[128, N])
                                    for ic in range(2):
                                        nc.tensor.matmul(
                                            ot[(c, ic)][:],
                                            v1s[jt][:, h * 33:(h + 1) * 33],
                                            pt_[:, ic * 512:(ic + 1) * 512],
                                            start=jt == 0, stop=jt == 7)
                            # denominators -> reciprocal rows, copy outT to SBUF
                            rrow = {}
                            ots = {}
                            for cl, c in enumerate(cs):
                                rw = RR2P.tile([1, N], F32, tag=f"rr{cl}", name="rw")
                                rwt = WK.tile([1, N], F32, tag="rwt", name="rwt")
                                for ic in range(2):
                                    sl = slice(ic * 512, (ic + 1) * 512)
                                    nc.scalar.activation(rwt[:, sl],
                                                         ot[(c, ic)][32:33, :], AF.Copy)
                                    o_s = OTS.tile([32, 512], F32, tag="otsb", name="otsb")
                                    nc.vector.tensor_copy(o_s[:], ot[(c, ic)][0:32, :])
                                    ots[(c, ic)] = o_s
                                nc.vector.reciprocal_approx_fast(rw[:], rwt[:])
                                rrow[c] = rw
                        with tc.tile_pool(name="psr2", bufs=2, space=PSUM) as PSR2:
                            for cl, c in enumerate(cs):
                                R1 = PSR2.tile([32, N], F32, tag="R1", name="R1")
                                for ic in range(2):
                                    sl = slice(ic * 512, (ic + 1) * 512)
                                    nc.tensor.matmul(R1[:, sl], o32r[:], rrow[c][:, sl],
                                                     start=True, stop=True)
                                for ic in range(2):
                                    sl = slice(ic * 512, (ic + 1) * 512)
                                    nc.vector.tensor_tensor(
                                        nss[g][c * 32:(c + 1) * 32, sl],
                                        ots[(c, ic)][:],
                                        R1[:, sl], op=OP.mult)

                for g in range(2):
                    dump("ns", g, nss[g][:], cast=[128, N])

                # ---------- phase 7: output projection ----------
                with tc.tile_pool(name="ps7", bufs=4, space=PSUM) as PS7:
                    for it in range(8):
                        ps = PS7.tile([128, DIM], F32, tag="psy")
                        for g in range(2):
                            nc.tensor.matmul(ps[:], nss[g][:, it * 128:(it + 1) * 128],
                                             Wos[g][:], start=g == 0, stop=g == 1)
                        yo = OTS.tile([128, DIM], F32, tag="yo", name="yo")
                        nc.vector.tensor_tensor(yo[:], ps[:], boutb[:], op=OP.add)
                        nc.sync.dma_start(y_d.ap()[it], yo[:])

            if reps == 1:
                phases()
            else:
                E = mybir.EngineType
                with tc.For_i(0, reps, 1,
                              hint_engines=(E.PE, E.DVE, E.Activation, E.Pool, E.SP)):
                    phases()
    return nc


def _make_eptx():
    bf = ml_dtypes.bfloat16
    ept = np.stack([(np.arange(32)[:, None] == (pt * 4 + np.arange(128)[None, :] // 32))
                    for pt in range(8)]).astype(bf).transpose(1, 0, 2).reshape(32, 8 * 128)
    epx = (np.arange(32)[:, None] == (np.arange(128)[None, :] % 32)).astype(bf)
    comb = np.zeros((64, 1152), bf)
    comb[0:32, 0:1024] = ept
    comb[32:64, 1024:1152] = epx
    return comb


def _tent_mat():
    d = np.arange(63)[:, None]
    p = np.arange(63)[None, :]
    pos = 31.0 + 0.96875 * (d - 31)
    return np.maximum(0.0, 1.0 - np.abs(pos - p)).astype(np.float32)


def prep_consts(Wq, Wkv, conv_w, conv_b, ln_g, ln_b, Woff, rpe, Wout, bout):
    bf = ml_dtypes.bfloat16
    scale = np.float32(32 ** -0.5)
    WY = _tent_mat()
    T = np.einsum('ap,hpq,bq->hab', WY, rpe.astype(np.float32), WY)
    expT = np.exp(T).astype(np.float32)  # [8, 63, 63]
    # expanded bias table: ebx[h, jt, p, i] = expT[h, 31+iy-jy, 31+ix-jx]
    # with j = jt*128+p, (jy,jx) = divmod(j,32), (iy,ix) = divmod(i,32)
    j = np.arange(N)
    i = np.arange(N)
    jy, jx = j // 32, j % 32
    iy, ix = i // 32, i % 32
    A = 31 + iy[None, :] - jy[:, None]   # [N, N] y-displacement index
    B = 31 + ix[None, :] - jx[:, None]   # [N, N] x-displacement index
    ebx = np.empty((8, N, N), bf)
    for h in range(8):
        ebx[h] = expT[h][A, B].astype(bf)
    ebx = ebx.reshape(8, 8, 128, N)
    jyf = (np.arange(N) // 32).astype(np.float32)
    jxf = (np.arange(N) % 32).astype(np.float32)
    return dict(
        Wq=Wq.astype(bf).reshape(2, 128, DIM),
        Wk=(Wkv[:, :DIM] * scale).astype(bf).reshape(2, 128, DIM),
        Wv=Wkv[:, DIM:].astype(bf).reshape(2, 128, DIM),
        Wo=Wout.astype(bf).reshape(2, 128, DIM),
        Woff=Woff.astype(np.float32),
        diagw=(np.eye(128, dtype=np.float32)[None, :, :]
               * conv_w.reshape(128, 25).astype(np.float32).T[:, :, None]
               ).transpose(1, 0, 2).reshape(128, 25 * 128).astype(bf),
        convb=conv_b.reshape(128, 1).astype(np.float32),
        cbmean=np.array([[conv_b.mean()]], np.float32),
        lng=ln_g.reshape(128, 1).astype(np.float32),
        lnb=ln_b.reshape(128, 1).astype(np.float32),
        npy64=np.concatenate([-np.arange(32, dtype=np.float32)] * 2).reshape(64, 1),
        Eptx=_make_eptx(),
        tyxb=np.stack([(jyf + 0.5) * 0.96875, (jxf + 0.5) * 0.96875]).astype(np.float32),
        inv128=np.full((128, 1), 1.0 / 128, np.float32),
        ones32r=np.ones((1, 32), np.float32),
        ones128r=np.ones((1, 128), np.float32),
        sel2=(np.arange(2)[:, None] == (np.arange(64)[None, :] // 32)).astype(np.float32),
        ebx=ebx,
        boutb=np.tile(bout.reshape(1, DIM), (128, 1)).astype(np.float32),
    )


def prep_x(x):
    """x [8, N, DIM] f32 -> concat xT [8*2, 128, N] bf16 (sharded by core)."""
    bf = ml_dtypes.bfloat16
    # [B, N, DIM] -> [B, DIM, N] -> [B, 2, 128, N] -> [B*2, 128, N]
    return np.ascontiguousarray(x.transpose(0, 2, 1)).astype(bf).reshape(8 * 2, 128, N)


_CACHE = {}


def _weights_key(ws):
    hsh = hashlib.sha1()
    for w in ws:
        hsh.update(np.ascontiguousarray(w).tobytes())
    return hsh.hexdigest()


def make_sharded(consts, reps=1):
    """Build + compile the Bass module and wrap it in an 8-core sharded jit."""
    import jax
    from concourse.bass2jax import (_bass_exec_p, install_neuronx_cc_hook,
                                    partition_id_tensor)
    from jax.sharding import Mesh, PartitionSpec
    from jax.experimental.shard_map import shard_map

    install_neuronx_cc_hook()
    nc = bacc.Bacc("TRN2", target_bir_lowering=False, debug=False)
    build(nc, consts, reps=reps)
    nc.compile()
    n_cores = 8
    in_names, out_names, out_avals, zero_outs = [], [], [], []
    for alloc in nc.m.functions[0].allocations:
        if not isinstance(alloc, mybir.MemoryLocationSet):
            continue
        nm = alloc.memorylocations[0].name
        if alloc.kind == "ExternalInput":
            if nc.partition_id_tensor is None or nm != nc.partition_id_tensor.name:
                in_names.append(nm)
        elif alloc.kind == "ExternalOutput":
            out_names.append(nm)
            shape = tuple(alloc.tensor_shape)
            dtype = mybir.dt.np(alloc.dtype)
            out_avals.append(jax.core.ShapedArray(shape, dtype))
            zero_outs.append(np.zeros((n_cores * shape[0], *shape[1:]), dtype))
    assert set(in_names) == {"xT", "ebx"}, in_names
    all_names = in_names + out_names
    if nc.partition_id_tensor is not None:
        all_names.append(nc.partition_id_tensor.name)

    def _body(*args):
        operands = list(args)
        if nc.partition_id_tensor is not None:
            operands.append(partition_id_tensor())
        return tuple(_bass_exec_p.bind(
            *operands, out_avals=tuple(out_avals), in_names=tuple(all_names),
            out_names=tuple(out_names), lowering_input_output_aliases=(),
            sim_require_finite=True, sim_require_nnan=True, nc=nc))

    devices = jax.devices()[:n_cores]
    mesh = Mesh(np.asarray(devices), ("core",))
    # ebx is identical on every core: pass it replicated (no 8x concat)
    in_sp = tuple(PartitionSpec() if nm == "ebx" else PartitionSpec("core")
                  for nm in in_names)
    sharded = jax.jit(
        shard_map(_body, mesh=mesh,
                  in_specs=in_sp + (PartitionSpec("core"),) * len(out_names),
                  out_specs=(PartitionSpec("core"),) * len(out_names),
                  check_rep=False),
        keep_unused=True)
    return sharded, in_names, out_names, zero_outs


def kernel(x, Wq, Wkv, conv_w, conv_b, ln_g, ln_b, Woff, rpe, Wout, bout):
    import jax
    from jax.sharding import Mesh, PartitionSpec, NamedSharding
    ws = (Wq, Wkv, conv_w, conv_b, ln_g, ln_b, Woff, rpe, Wout, bout)
    wkey = _weights_key(ws)
    if _CACHE.get('wkey') != wkey:
        consts = prep_consts(*[np.asarray(w) for w in ws])
        _CACHE['sharded'] = make_sharded(consts)
        mesh = Mesh(np.asarray(jax.devices()[:8]), ("core",))
        sh = NamedSharding(mesh, PartitionSpec())
        _CACHE['ebx_dev'] = jax.device_put(consts['ebx'], sh)
        _CACHE['wkey'] = wkey
    sharded, in_names, out_names, zero_outs = _CACHE['sharded']
    xT = prep_x(np.asarray(x))
    args = [xT if nm == 'xT' else _CACHE['ebx_dev'] for nm in in_names]
    outs = sharded(*args, *zero_outs)
    y = np.asarray(outs[out_names.index("y")])
    return y.reshape(8, 8, 128, DIM).reshape(8, N, DIM).astype(np.float32)


if __name__ == '__main__':
    rng = np.random.default_rng(0)
    ins = dict(
        x=rng.standard_normal((8, N, DIM), np.float32),
        Wq=rng.standard_normal((DIM, DIM), np.float32) * 0.02,
        Wkv=rng.standard_normal((DIM, 2 * DIM), np.float32) * 0.02,
        conv_w=rng.standard_normal((128, 1, 5, 5), np.float32) * 0.02,
        conv_b=np.zeros(128, np.float32),
        ln_g=np.ones(128, np.float32),
        ln_b=np.zeros(128, np.float32),
        Woff=rng.standard_normal((128, 2), np.float32) * 0.02,
        rpe=rng.standard_normal((8, 63, 63), np.float32) * 0.01,
        Wout=rng.standard_normal((DIM, DIM), np.float32) * 0.02,
        bout=np.zeros(DIM, np.float32),
    )
    y = kernel(**ins)
    print('out', y.shape, y.dtype, float(np.abs(y).max()))
